# revision 1
# baseline (speedup 1.0000x reference)
"""GraphSAGE on 8 TRN2 cores — sharded-h1 formulation.

Layer 1 (h1) is computed exactly once globally, only at nodes actually
referenced by the output (batch ∪ neigh(batch), ~75K unique of 100K),
sharded across the 8 cores. The per-core h1 shards are AllGathered into a
replicated h1 table, then each core computes layer 2 + projection for its
1024 batch rows by gathering h1 rows. This cuts random-row DMA descriptors
(the hard bottleneck on this part: ~190ns/descriptor/engine) from ~296K to
~178K per core vs the owner-computes-batch formulation.
"""

import numpy as np

import concourse.bacc as bacc
import concourse.bass as bass
import concourse.mybir as mybir
import concourse.tile as tile
from concourse.bass_utils import run_bass_kernel_spmd
from concourse.masks import make_identity
from contextlib import ExitStack

N_NODES = 100000
D = 128
S = 16
BATCH = 8192
N_CORES = 8
NB = BATCH // N_CORES            # 1024 batch rows per core
P = 128
SELF_TILES = NB // P             # 8 phase-2 tiles per core

F32 = mybir.dt.float32
I32 = mybir.dt.int32


def build_program(n_tiles1):
    """n_tiles1: phase-1 tiles per core (shard size = n_tiles1*128 nodes)."""
    mp8 = n_tiles1 * P
    mp = mp8 * N_CORES
    nc = bacc.Bacc("TRN2", target_bir_lowering=False, debug=False,
                   enable_asserts=False, num_devices=N_CORES)

    x_d = nc.dram_tensor("x", [N_NODES, D], F32, kind="ExternalInput").ap()
    g1_d = nc.dram_tensor("g1", [mp8, S + 1], I32, kind="ExternalInput").ap()
    g2_d = nc.dram_tensor("g2", [NB, S + 1], I32, kind="ExternalInput").ap()
    w1a_d = nc.dram_tensor("w1a", [P, P], F32, kind="ExternalInput").ap()
    w1b_d = nc.dram_tensor("w1b", [P, P], F32, kind="ExternalInput").ap()
    w2a_d = nc.dram_tensor("w2a", [P, P], F32, kind="ExternalInput").ap()
    w2b_d = nc.dram_tensor("w2b", [P, P], F32, kind="ExternalInput").ap()
    wout_d = nc.dram_tensor("wout", [P, 64], F32, kind="ExternalInput").ap()
    b1_d = nc.dram_tensor("b1", [P, 1], F32, kind="ExternalInput").ap()
    b2_d = nc.dram_tensor("b2", [P, 1], F32, kind="ExternalInput").ap()
    bout_d = nc.dram_tensor("bout", [64, 1], F32, kind="ExternalInput").ap()
    out_d = nc.dram_tensor("out", [NB, 64], F32, kind="ExternalOutput").ap()

    half = (n_tiles1 // 2) * P
    h1_mine = nc.dram_tensor("h1_mine", [mp8, D], F32)   # internal
    h1_full = nc.dram_tensor("h1_full", [mp, D], F32)    # internal, post-AllGather

    with tile.TileContext(nc) as tc, ExitStack() as ctx:
        consts = ctx.enter_context(tc.tile_pool(name="consts", bufs=1))
        idx_pool = ctx.enter_context(tc.tile_pool(name="idx", bufs=4))
        wide_pool = ctx.enter_context(tc.tile_pool(name="wide", bufs=3))
        sb_pool = ctx.enter_context(tc.tile_pool(name="sb", bufs=3))
        ps = ctx.enter_context(tc.tile_pool(name="ps", bufs=2, space="PSUM"))

        w1a = consts.tile([P, P], F32, tag="w1a")
        w1b = consts.tile([P, P], F32, tag="w1b")
        w2a = consts.tile([P, P], F32, tag="w2a")
        w2b = consts.tile([P, P], F32, tag="w2b")
        wout = consts.tile([P, 64], F32, tag="wout")
        b1 = consts.tile([P, 1], F32, tag="b1")
        b2 = consts.tile([P, 1], F32, tag="b2")
        bout = consts.tile([64, 1], F32, tag="bout")
        ident = consts.tile([P, P], F32, tag="ident")
        for t_sb, t_d in ((w1a, w1a_d), (w1b, w1b_d), (w2a, w2a_d), (w2b, w2b_d),
                          (wout, wout_d), (b1, b1_d), (b2, b2_d), (bout, bout_d)):
            nc.sync.dma_start(out=t_sb[:], in_=t_d[:, :])
        make_identity(nc, ident[:])

        def sage_tile(idx_src, gather_tbl, wa, wb, bias, act_fn):
            """One 128-node SAGE tile: gather own+16 neighbors, max-agg,
            transpose, two matmuls, bias+activation. Returns hT sbuf tile
            [128 featout, 128 nodes] (feature-major)."""
            idx = idx_pool.tile([P, S + 1], I32, tag="idx")
            nc.sync.dma_start(out=idx[:], in_=idx_src)
            wide = wide_pool.tile([P, (S + 1) * D], F32, tag="wide")
            for s in range(S + 1):
                nc.gpsimd.indirect_dma_start(
                    out=wide[:, s * D:(s + 1) * D],
                    out_offset=None,
                    in_=gather_tbl,
                    in_offset=bass.IndirectOffsetOnAxis(ap=idx[:, s:s + 1], axis=0),
                )
            agg = sb_pool.tile([P, D], F32, tag="agg")
            wide3 = wide[:].rearrange("p (s f) -> p f s", f=D)
            nc.vector.reduce_max(agg[:], wide3[:, :, 1:], axis=mybir.AxisListType.X)

            ownT_ps = ps.tile([P, P], F32, tag="ownT_ps", space="PSUM")
            nc.tensor.transpose(ownT_ps[:], wide[:, 0:D], ident[:])
            aggT_ps = ps.tile([P, P], F32, tag="aggT_ps", space="PSUM")
            nc.tensor.transpose(aggT_ps[:], agg[:], ident[:])
            ownT = sb_pool.tile([P, P], F32, tag="ownT")
            nc.scalar.copy(out=ownT[:], in_=ownT_ps[:])
            aggT = sb_pool.tile([P, P], F32, tag="aggT")
            nc.scalar.copy(out=aggT[:], in_=aggT_ps[:])

            hps = ps.tile([P, P], F32, tag="hps", space="PSUM")
            nc.tensor.matmul(hps[:], lhsT=wa[:], rhs=ownT[:], start=True, stop=False)
            nc.tensor.matmul(hps[:], lhsT=wb[:], rhs=aggT[:], start=False, stop=True)
            hT = sb_pool.tile([P, P], F32, tag="hT")
            nc.scalar.activation(hT[:], hps[:], act_fn, bias=bias[:, :1])
            return hT

        # ---- phase 1: h1 at this core's shard of referenced nodes ----
        for t in range(n_tiles1):
            h1T = sage_tile(g1_d[t * P:(t + 1) * P, :], x_d[:, :], w1a, w1b, b1,
                            mybir.ActivationFunctionType.Relu)
            nm_ps = ps.tile([P, P], F32, tag="nm_ps", space="PSUM")
            nc.tensor.transpose(nm_ps[:], h1T[:], ident[:])
            h1nm = sb_pool.tile([P, P], F32, tag="h1nm")
            nc.vector.tensor_copy(out=h1nm[:], in_=nm_ps[:])
            nc.sync.dma_start(out=h1_mine.ap()[t * P:(t + 1) * P, :], in_=h1nm[:])
            if (t + 1) * P == half:
                # first-half AllGather overlaps the rest of phase 1
                nc.gpsimd.collective_compute(
                    "AllGather",
                    mybir.AluOpType.bypass,
                    replica_groups=[list(range(N_CORES))],
                    ins=[h1_mine.ap()[0:half, :].opt()],
                    outs=[h1_full.ap()[0:half * N_CORES, :].opt()],
                )

        nc.gpsimd.collective_compute(
            "AllGather",
            mybir.AluOpType.bypass,
            replica_groups=[list(range(N_CORES))],
            ins=[h1_mine.ap()[half:mp8, :].opt()],
            outs=[h1_full.ap()[half * N_CORES:mp, :].opt()],
        )

        # ---- phase 2: layer 2 + projection for this core's batch rows ----
        for g in range(SELF_TILES):
            h2T = sage_tile(g2_d[g * P:(g + 1) * P, :], h1_full.ap()[:, :],
                            w2a, w2b, b2, mybir.ActivationFunctionType.Identity)
            ops = ps.tile([64, P], F32, tag="ownT_ps", space="PSUM")
            nc.tensor.matmul(ops[:], lhsT=wout[:], rhs=h2T[:], start=True, stop=True)
            outT = sb_pool.tile([64, P], F32, tag="outT")
            nc.scalar.activation(outT[:], ops[:],
                                 mybir.ActivationFunctionType.Identity,
                                 bias=bout[:, :1])
            trps = ps.tile([P, 64], F32, tag="aggT_ps", space="PSUM")
            nc.tensor.transpose(trps[:], outT[:], ident[:64, :64])
            outsb = sb_pool.tile([P, 64], F32, tag="outsb")
            nc.vector.tensor_copy(out=outsb[:], in_=trps[:])
            nc.sync.dma_start(out=out_d[g * P:(g + 1) * P, :], in_=outsb[:])

    return nc


_CACHE = {}


def _get_compiled(n_tiles1):
    if _CACHE.get("n_tiles1") != n_tiles1:
        nc = build_program(n_tiles1)
        nc.compile()
        _CACHE["nc"] = nc
        _CACHE["n_tiles1"] = n_tiles1
    return _CACHE["nc"]


def make_in_maps(x, neigh_idx, batch, W1, b1, W2, b2, Wout, bout):
    x = np.ascontiguousarray(np.asarray(x, dtype=np.float32))
    ni = np.asarray(neigh_idx, dtype=np.int64)
    bt = np.asarray(batch, dtype=np.int64)
    W1 = np.asarray(W1, dtype=np.float32)
    W2 = np.asarray(W2, dtype=np.float32)

    ref_nodes = np.unique(np.concatenate([bt, ni[bt].ravel()]))
    M = len(ref_nodes)
    mp8 = -(-M // (N_CORES * P)) * P           # per-core shard, mult of 128
    mp = mp8 * N_CORES
    padded = np.concatenate([ref_nodes,
                             np.zeros(mp - M, dtype=ref_nodes.dtype)])
    half = (mp8 // P // 2) * P
    pos = np.arange(mp)
    c, r = pos // mp8, pos % mp8
    full_pos = np.where(r < half, c * half + r,
                        N_CORES * half + c * (mp8 - half) + (r - half))
    glob2loc = np.zeros(N_NODES, dtype=np.int64)
    glob2loc[ref_nodes] = full_pos[:M]

    common = {
        "x": x,
        "w1a": np.ascontiguousarray(W1[:P]),
        "w1b": np.ascontiguousarray(W1[P:]),
        "w2a": np.ascontiguousarray(W2[:P]),
        "w2b": np.ascontiguousarray(W2[P:]),
        "wout": np.ascontiguousarray(np.asarray(Wout, np.float32)),
        "b1": np.ascontiguousarray(np.asarray(b1, np.float32).reshape(P, 1)),
        "b2": np.ascontiguousarray(np.asarray(b2, np.float32).reshape(P, 1)),
        "bout": np.ascontiguousarray(np.asarray(bout, np.float32).reshape(64, 1)),
    }
    in_maps = []
    for c in range(N_CORES):
        own1 = padded[c * mp8:(c + 1) * mp8]
        g1 = np.concatenate([own1[:, None], ni[own1]], axis=1)
        bc = bt[c * NB:(c + 1) * NB]
        g2 = np.concatenate([glob2loc[bc][:, None], glob2loc[ni[bc]]], axis=1)
        in_maps.append(dict(common,
                            g1=np.ascontiguousarray(g1.astype(np.int32)),
                            g2=np.ascontiguousarray(g2.astype(np.int32))))
    return in_maps, mp8 // P


def run(in_maps, n_tiles1, trace=False, **kw):
    nc = _get_compiled(n_tiles1)
    return run_bass_kernel_spmd(nc, in_maps, core_ids=list(range(N_CORES)),
                                trace=trace, **kw)


def kernel(x, neigh_idx, batch, W1, b1, W2, b2, Wout, bout):
    in_maps, n_tiles1 = make_in_maps(x, neigh_idx, batch, W1, b1, W2, b2,
                                     Wout, bout)
    res = run(in_maps, n_tiles1)
    outs = [np.asarray(res.results[c]["out"]) for c in range(N_CORES)]
    return np.concatenate(outs, axis=0).astype(np.float32)

